# revision 29
# baseline (speedup 1.0000x reference)
"""Trainium2 Bass kernel for nn_Attention_3264175145451.

Full (unsharded) inputs in, full output out. Data-parallel over batch:
16 images / 8 cores = 2 images per core, no collectives. ~264us HW
(v1 baseline: 285us).

Design notes:
  - Host-side layout prep: x ships pre-transposed + pre-cast to bf16
    (xT, two contiguous half chunks) -> no PE transposes, no weight-cast
    chain; first matmul at ~8us instead of 26us. Residual + b_out fold
    into a second host tensor xres (f32), prefetched whole at startup.
  - K^T stored as head pairs [128, 4, n] (even head partitions 0-63, odd
    64-127, exactly as the projection emits it); QK^T runs K=64 matmuls
    at base partition 0/64 - no zero padding or memsets.
  - QK^T i-halves share one [128,1024] PSUM tile so exp is ONE ScalarE
    activation per (head, jt): (1024+352)/1.2ns each; ACT busy ~138us
    is the soft floor, PE ~totals 210us and paces the kernel.
  - PSUM: "mm" 2x[128,1024] (QK + tail) + "pp" 1x[128,1024] (fillers) +
    "o" 1x[65,1024] (AV accumulator) = 8 banks exactly.
  - ALL engine queues are strict in-order FIFOs, so the schedule lives in
    the emission order: projection/prep work is chopped into per-matmul
    generators pulled one matmul at a time inside the attention jt-loop
    (fills the PE's exp-wait gaps without head-of-line blocking).
  - Softmax normalization is split into three pipelined phases so no
    engine FIFO ever stalls on its DMA chain: (a) right after AV, DVE
    copies O^T and the denominator row out of PSUM (frees the PSUM slot)
    and the s->[64,16] DRAM bounce starts; (b) one head later, DVE
    reciprocal + broadcast bounce; (c) the normalize multiply runs on the
    otherwise-idle GpSimd whose FIFO absorbs the broadcast-DMA wait.
  - PE_HAM drops the PE clock 2.4->1.2 GHz whenever matmul density sags;
    throwaway matmuls pre-warm the clock at startup, idle-filler dummies
    hold it through filler-dry heads, and a dummy burst covers the final
    exposed norm chain. The last head processed is even-parity (direct
    ot write) and out_proj(1) accumulates per head-pair in SBUF so only
    its final quarter sits in the tail.
"""

import os
import sys

sys.path.insert(0, "/opt/trn_rl_repo")

import numpy as np
import ml_dtypes

import concourse.bass as bass  # noqa: F401
import concourse.mybir as mybir
import concourse.tile as tile
from concourse import bacc
from concourse.bass_utils import run_bass_kernel_spmd

F32 = mybir.dt.float32
BF = mybir.dt.bfloat16
AF = mybir.ActivationFunctionType
OP = mybir.AluOpType

B = 16           # total batch
NB = 2           # batches per core
N = 1024         # tokens per image (32*32)
C = 512          # channels
H = 8            # heads
D = 64           # head dim
NCORES = 8

TRACE = bool(int(os.environ.get("BASS_ATTN_TRACE", "0")))

_cache = {}


def _register_ntff_hook():
    """Register the axon NTFF profile hook if the image lacks antenv.axon_hooks."""
    import types

    try:
        from antenv.axon_hooks import get_axon_ntff_profile_hook  # noqa: F401
        return
    except ImportError:
        pass
    try:
        from trn_agent_boot.trn_boot import _ntff_profile_via_ctypes

        hook = _ntff_profile_via_ctypes("/opt/axon/libaxon_pjrt.so")
        mod = types.ModuleType("antenv.axon_hooks")
        mod.get_axon_ntff_profile_hook = lambda: hook
        sys.modules["antenv.axon_hooks"] = mod
    except Exception:
        pass


def build_nc():
    nc = bacc.Bacc("TRN2", target_bir_lowering=False, debug=False,
                   num_devices=NCORES)

    xT_ext = nc.dram_tensor("xT", [NB, 2, 128, 4, C], BF, kind="ExternalInput").ap()
    xres_ext = nc.dram_tensor("xres", [NB, N, C], F32, kind="ExternalInput").ap()
    wq_ext = nc.dram_tensor("wq", [128, 4, C], BF, kind="ExternalInput").ap()
    wk_ext = nc.dram_tensor("wk", [128, 4, C], BF, kind="ExternalInput").ap()
    wv_ext = nc.dram_tensor("wv", [128, 4, C], BF, kind="ExternalInput").ap()
    wo_ext = nc.dram_tensor("wo", [128, 4, C], BF, kind="ExternalInput").ap()
    bq_ext = nc.dram_tensor("bq", [128, 4], F32, kind="ExternalInput").ap()
    bk_ext = nc.dram_tensor("bk", [128, 4], F32, kind="ExternalInput").ap()
    bv_ext = nc.dram_tensor("bv", [C], F32, kind="ExternalInput").ap()
    y_ext = nc.dram_tensor("y", [NB, N, C], F32, kind="ExternalOutput").ap()

    with tile.TileContext(nc) as tc:
        _body(nc, tc, xT_ext, xres_ext, wq_ext, wk_ext, wv_ext, wo_ext,
              bq_ext, bk_ext, bv_ext, y_ext)
    nc.finalize()
    return nc


def _body(nc, tc, xT_ext, xres_ext, wq_ext, wk_ext, wv_ext, wo_ext,
          bq_ext, bk_ext, bv_ext, y_ext):
    from contextlib import ExitStack

    ctx = ExitStack()
    with ctx:
        wp = ctx.enter_context(tc.tile_pool(name="wp", bufs=1))
        persist = ctx.enter_context(tc.tile_pool(name="persist", bufs=2))
        ep = ctx.enter_context(tc.tile_pool(name="ep", bufs=4))
        rp = ctx.enter_context(tc.tile_pool(name="rp", bufs=2))
        rbp = ctx.enter_context(tc.tile_pool(name="rbp", bufs=2))
        tbp = ctx.enter_context(tc.tile_pool(name="tbp", bufs=2))
        xrp = ctx.enter_context(tc.tile_pool(name="xrp", bufs=8))
        yp = ctx.enter_context(tc.tile_pool(name="yp", bufs=3))
        yaccp = ctx.enter_context(tc.tile_pool(name="yacc", bufs=1))
        drp = ctx.enter_context(tc.tile_pool(name="drp", bufs=4, space="DRAM"))
        psq = ctx.enter_context(tc.tile_pool(name="psq", bufs=2, space="PSUM"))
        pso = ctx.enter_context(tc.tile_pool(name="pso", bufs=1, space="PSUM"))

        def ps_tile(tag, name):
            return psq.tile([128, 2 * C], F32, tag=tag,
                            bufs=(2 if tag == "mm" else 1), name=name)

        # ---- weights / biases (host-prepared layouts, tiny DMAs) ----
        wq_sb = wp.tile([128, 4, C], BF, tag="wq")
        wk_sb = wp.tile([128, 4, C], BF, tag="wk")
        wv_sb = wp.tile([128, 4, C], BF, tag="wv")
        wo_sb = wp.tile([128, 4, C], BF, tag="wo")
        bq_col = wp.tile([128, 4], F32, tag="bqc")
        bk_col = wp.tile([128, 4], F32, tag="bkc")
        bv_bc = wp.tile([128, C], F32, tag="bvb")
        def alloc_tiles():
            # xT layout: [p, ih-half, kt, n-within-half] so each half is one
            # contiguous DMA (4KB per-partition rows)
            xT = persist.tile([128, 2, 4, C], BF, tag="xT")
            q_sb = persist.tile([128, 4, N], BF, tag="q")
            k_sb = persist.tile([128, 4, N], BF, tag="k")
            v_sb = persist.tile([128, 8, H, D + 1], BF, tag="v")
            ot = persist.tile([128, 4, N], BF, tag="ot")
            return xT, q_sb, k_sb, v_sb, ot

        tiles0 = alloc_tiles()
        # startup: xT(0) first (two contiguous chunks so V-proj starts early),
        # weights on the scalar HWDGE queue (ACT idle here) in use order.
        nc.sync.dma_start(out=tiles0[0][:, 0], in_=xT_ext[0, 0])
        nc.scalar.dma_start(out=wv_sb[:], in_=wv_ext)
        nc.sync.dma_start(out=wq_sb[:], in_=wq_ext)
        nc.sync.dma_start(out=wk_sb[:], in_=wk_ext)
        nc.sync.dma_start(out=tiles0[0][:, 1], in_=xT_ext[0, 1])
        nc.scalar.dma_start(out=bq_col[:], in_=bq_ext)
        nc.scalar.dma_start(out=bk_col[:], in_=bk_ext)
        # b_v broadcast over partitions with a 0-stride source AP
        nc.scalar.dma_start(
            out=bv_bc[:],
            in_=bass.AP(tensor=bv_ext.tensor, offset=0, ap=[[0, 128], [1, C]]))
        nc.scalar.dma_start(out=wo_sb[:], in_=wo_ext)

        # HAM pre-warm: ~4.5us of throwaway matmuls on a memset scratch tile
        # while the first DMAs land, so real matmuls start at 2.4 GHz.
        scratch = wp.tile([128, C], BF, tag="scratch")
        nc.gpsimd.memset(scratch[:], 0.0)
        pwarm = ps_tile("mm", "pwarm")
        for _ in range(18):
            nc.tensor.matmul(pwarm[:, 0:C], scratch[:, 0:128], scratch[:],
                             start=True, stop=True)

        def prep_start(b, tiles, skip_xt=False):
            xT, q_sb, k_sb, v_sb, ot = tiles
            if not skip_xt:
                nc.sync.dma_start(
                    out=xT[:],
                    in_=xT_ext[b].rearrange("c p k n -> p c k n"))
            # ones column of V (softmax denominator trick)
            nc.gpsimd.memset(v_sb[:, :, :, D:D + 1], 1.0)

        def prep_factories(tiles):
            """Generator factories (one PSUM tile each, yield per matmul)."""
            xT, q_sb, k_sb, v_sb, ot = tiles

            def vproj(pr):
                def gen(tag):
                    pv = ps_tile(tag, "pv")
                    pv_v = pv[:].rearrange("p (two h d) -> p two h d",
                                           two=2, h=H)
                    for q in range(2):
                        it = 2 * pr + q
                        off = (it % 4) * 128
                        for kt in range(4):
                            nc.tensor.matmul(
                                pv[:, bass.ts(q, C)],
                                xT[:, it // 4, kt, off:off + 128],
                                wv_sb[:, kt, :],
                                start=(kt == 0), stop=(kt == 3))
                            yield
                        nc.vector.tensor_tensor(
                            v_sb[:, 2 * pr + q, :, 0:D],
                            pv_v[:, q],
                            bv_bc[:].rearrange("p (h d) -> p h d", h=H),
                            op=OP.add)
                return gen

            def qkproj(mt, w_sb, b_col, dst):
                def gen(tag):
                    pt = ps_tile(tag, "pt")
                    for ih in range(2):
                        for kt in range(4):
                            nc.tensor.matmul(
                                pt[:, bass.ts(ih, C)],
                                w_sb[:, kt, bass.ts(mt, 128)],
                                xT[:, ih, kt, :],
                                start=(kt == 0), stop=(kt == 3))
                            yield
                    nc.vector.tensor_scalar_add(
                        dst[:, mt, :], pt[:], b_col[:, mt:mt + 1])
                return gen

            tasks = [vproj(0), qkproj(0, wq_sb, bq_col, q_sb),
                     qkproj(0, wk_sb, bk_col, k_sb),
                     vproj(1), qkproj(1, wq_sb, bq_col, q_sb),
                     qkproj(1, wk_sb, bk_col, k_sb),
                     vproj(2), vproj(3),
                     qkproj(2, wq_sb, bq_col, q_sb),
                     qkproj(2, wk_sb, bk_col, k_sb),
                     qkproj(3, wq_sb, bq_col, q_sb),
                     qkproj(3, wk_sb, bk_col, k_sb)]
            return tasks

        # ---- fine-grained filler queue: one matmul per pull, interleaved
        # into the attention jt-loop so the PE never idles on ACT waits ----
        from collections import deque
        fillq = deque()

        def add_fillers(gens):
            for g in gens:
                fillq.append(g("pp"))

        def pull(k=1, idle_dummy=False):
            n = 0
            while fillq and n < k:
                try:
                    next(fillq[0])
                    n += 1
                except StopIteration:
                    fillq.popleft()
            if n == 0 and idle_dummy:
                dm = ps_tile("mm", "dm")
                nc.tensor.matmul(dm[:, 0:256], scratch[:, 0:128],
                                 scratch[:, 0:256], start=True, stop=True)

        def norm_a(g, parity, pso_t, ot, fast=False):
            """Right after AV: pull O^T and s out of PSUM (frees the pso
            slot) and kick off the s -> [64,16] DRAM bounce. No DVE op here
            ever waits on a DMA."""
            o_sb = tbp.tile([64, N], BF, tag="tb")
            nc.vector.tensor_copy(o_sb[:], pso_t[0:D, :])
            s_row = rp.tile([128, N], F32, tag="r")
            nc.vector.tensor_copy(s_row[64:65, :], pso_t[D:D + 1, :])
            sd = drp.tile([N], F32, tag="sd")
            (nc.scalar if fast else nc.sync).dma_start(
                out=sd[:], in_=s_row[64:65, :])
            sp = rp.tile([64, 16], F32, tag="sp")
            (nc.scalar if fast else nc.gpsimd).dma_start(
                out=sp[:], in_=sd[:].rearrange("(p f) -> p f", p=64))
            return o_sb, sp

        def norm_b(g, parity, o_sb, sp, ot, fast=False):
            """One head later: reciprocal (its DMA input long done), the
            broadcast bounce, and the normalize multiply on GpSimd (whose
            idle FIFO absorbs the DMA wait instead of the DVE's). In fast
            (tail) mode the DMAs ride the idle scalar queue and the multiply
            runs on the now-idle DVE."""
            rsp = rp.tile([64, 16], F32, tag="rsp")
            nc.vector.reciprocal(out=rsp[:], in_=sp[:])
            rd = drp.tile([N], F32, tag="rd")
            (nc.scalar if fast else nc.sync).dma_start(
                out=rd[:].rearrange("(p f) -> p f", p=64), in_=rsp[:])
            _rd = rd[:]
            rb_t = rbp.tile([64, N], F32, tag="rb")
            (nc.scalar if fast else nc.gpsimd).dma_start(
                out=rb_t[:], in_=bass.AP(
                    tensor=_rd.tensor, offset=_rd.offset,
                    ap=[[0, 64], [1, N]]))
            eng = nc.vector if fast else nc.gpsimd
            if parity == 0:
                eng.tensor_tensor(ot[0:64, g, :], o_sb[:], rb_t[:],
                                  op=OP.mult)
            else:
                tb = tbp.tile([64, N], BF, tag="tb2")
                eng.tensor_tensor(tb[:], o_sb[:], rb_t[:], op=OP.mult)
                (nc.scalar if fast else nc.gpsimd).dma_start(
                    out=ot[64:128, g, :], in_=tb[:])

        pending_norm = [None]

        def head(tiles, hh, flush=False):
            """One attention head; filler matmuls pulled in per jt step."""
            xT, q_sb, k_sb, v_sb, ot = tiles
            g, parity = hh // 2, hh % 2
            base = 64 * parity
            pso_t = pso.tile([D + 1, N], F32, tag="o")
            for jt in range(8):
                spair = ps_tile("mm", "spair")
                for ih in range(2):
                    nc.tensor.matmul(
                        spair[:, bass.ts(ih, C)],
                        k_sb[base:base + 64, g, bass.ts(jt, 128)],
                        q_sb[base:base + 64, g, bass.ts(ih, C)],
                        start=True, stop=True)
                pull(1)
                e_t = ep.tile([128, N], BF, tag="E")
                nc.scalar.activation(out=e_t[:], in_=spair[:],
                                     func=AF.Exp, scale=0.125)
                for ih in range(2):
                    nc.tensor.matmul(pso_t[:, bass.ts(ih, C)],
                                     v_sb[:, jt, hh, :],
                                     e_t[:, bass.ts(ih, C)],
                                     start=(jt == 0), stop=(jt == 7))
                pull(1, idle_dummy=True)
            if pending_norm[0] is not None:
                pending_norm[0]()  # previous head's norm tail (DMAs long done)
            o_sb, sp = norm_a(g, parity, pso_t, ot, fast=flush)
            if flush:
                norm_b(g, parity, o_sb, sp, ot, fast=True)
                pending_norm[0] = None
            else:
                pending_norm[0] = lambda: norm_b(g, parity, o_sb, sp, ot)

        def prefetch_xres(b):
            """Load all of image b's residual tiles up front (4 x 512KB) so
            projection DVE adds never wait on DMA mid-attention."""
            tiles = []
            for pr in range(4):
                xr = xrp.tile([128, 2, C], F32, tag="xr", name="xr")
                nc.sync.dma_start(
                    out=xr[:],
                    in_=xres_ext[b, bass.ts(pr, 256), :].rearrange(
                        "(q p) c -> p q c", p=128))
                tiles.append(xr)
            return tiles

        def out_proj_psum(b, ot, xrs):
            """PSUM-accumulated projection (image 0), generator per pr."""
            def task(pr):
                def gen(tag):
                    py = ps_tile(tag, "py")
                    for q in range(2):
                        it = 2 * pr + q
                        for gg in range(4):
                            nc.tensor.matmul(
                                py[:, bass.ts(q, C)],
                                ot[:, gg, bass.ts(it, 128)],
                                wo_sb[:, gg, :],
                                start=(gg == 0), stop=(gg == 3))
                            yield
                    yt = yp.tile([128, 2, C], F32, tag="y")
                    nc.vector.tensor_tensor(
                        yt[:], py[:].rearrange("p (q c) -> p q c", q=2),
                        xrs[pr][:], op=OP.add)
                    nc.sync.dma_start(
                        out=y_ext[b, bass.ts(pr, 256), :].rearrange(
                            "(q p) c -> p q c", p=128),
                        in_=yt[:])
                return gen
            return [task(pr) for pr in range(4)]

        # ---- image 1's projection: per-head-pair SBUF accumulation so the
        # matmuls interleave into attention(1)'s tail heads ----
        def out_proj_sbuf_g(b, ot, yacc, gg, xrs, split=False):
            def gen(tag):
                for pr in range(4):
                    py = ps_tile(tag, "pyg")
                    if split:
                        for q in range(2):
                            it = 2 * pr + q
                            nc.tensor.matmul(
                                py[:, bass.ts(q, C)],
                                ot[0:64, gg, bass.ts(it, 128)],
                                wo_sb[0:64, gg, :],
                                start=True, stop=False)
                            yield
                        for q in range(2):
                            it = 2 * pr + q
                            nc.tensor.matmul(
                                py[:, bass.ts(q, C)],
                                ot[64:128, gg, bass.ts(it, 128)],
                                wo_sb[64:128, gg, :],
                                start=False, stop=True)
                            yield
                    else:
                        for q in range(2):
                            it = 2 * pr + q
                            nc.tensor.matmul(py[:, bass.ts(q, C)],
                                             ot[:, gg, bass.ts(it, 128)],
                                             wo_sb[:, gg, :],
                                             start=True, stop=True)
                            yield
                    if gg == 0:
                        nc.vector.tensor_tensor(
                            yacc[:, pr],
                            py[:].rearrange("p (q c) -> p q c", q=2),
                            xrs[pr][:], op=OP.add)
                    else:
                        nc.vector.tensor_tensor(
                            yacc[:, pr],
                            py[:].rearrange("p (q c) -> p q c", q=2),
                            yacc[:, pr], op=OP.add)
                    if gg == 3:
                        nc.sync.dma_start(
                            out=y_ext[b, bass.ts(pr, 256), :].rearrange(
                                "(q p) c -> p q c", p=128),
                            in_=yacc[:, pr])
            return gen

        # ================= schedule =================
        tiles1 = alloc_tiles()
        yacc = yaccp.tile([128, 4, 2, C], F32, tag="yacc")

        # prep(0) runs stand-alone; alternate the two PSUM rings for a
        # 2-deep pipeline (no attention is competing for them yet).
        prep_start(0, tiles0, skip_xt=True)
        for i, fac in enumerate(prep_factories(tiles0)):
            for _ in fac("mm" if i % 2 == 0 else "pp"):
                pass

        prep_start(1, tiles1)
        xrs0 = prefetch_xres(0)
        xrs1 = prefetch_xres(1)

        # attention(0): prep(1) matmuls drip in from head 1 (its xT DMA
        # needs ~20us to land; head 0 runs bare). 12 factories spread
        # 2,2,2,2,2,1,1 over heads 1-7.
        t1f = prep_factories(tiles1)
        head(tiles0, 0)
        counts = [2, 2, 2, 2, 2, 1, 1]
        for hh in range(1, 8):
            k = counts[hh - 1]
            add_fillers(t1f[:k])
            t1f = t1f[k:]
            head(tiles0, hh, flush=(hh == 7))
        assert not t1f and not fillq

        # attention(1): out_proj(0) one tile per even head; out_proj(1)
        # g-groups two heads after their pair's norm tail lands in ot.
        p0 = out_proj_psum(0, tiles0[4], xrs0)
        add_fillers([p0[0]])
        head(tiles1, 0)
        add_fillers([p0[1]])
        head(tiles1, 1)
        add_fillers([p0[2]])
        head(tiles1, 2)
        add_fillers([out_proj_sbuf_g(1, tiles1[4], yacc, 0, xrs1)])
        head(tiles1, 3)
        add_fillers([p0[3]])
        head(tiles1, 4)
        add_fillers([out_proj_sbuf_g(1, tiles1[4], yacc, 1, xrs1)])
        head(tiles1, 5)
        head(tiles1, 7)
        add_fillers([out_proj_sbuf_g(1, tiles1[4], yacc, 2, xrs1)])
        head(tiles1, 6, flush=True)
        # keep the PE clock warm across the exposed final norm chain
        for _ in range(72):
            dm = ps_tile("mm", "dm")
            nc.tensor.matmul(dm[:, 0:C], scratch[:, 0:128], scratch[:],
                             start=True, stop=True)
        for _ in out_proj_sbuf_g(1, tiles1[4], yacc, 3, xrs1)("mm"):
            pass
        assert not fillq


def _prep_inputs(x, w_qkv, b_qkv, w_out, b_out):
    bf16 = ml_dtypes.bfloat16
    x = np.ascontiguousarray(np.asarray(x, dtype=np.float32))
    w_qkv = np.asarray(w_qkv, dtype=np.float32)
    b_qkv = np.asarray(b_qkv, dtype=np.float32)
    w_out = np.asarray(w_out, dtype=np.float32)
    b_out = np.asarray(b_out, dtype=np.float32)

    x_flat = x.reshape(B, N, C)
    # xT[b, c, p, kt, n'] = x[b, c*512+n', kt*128+p], bf16 (half-chunks
    # contiguous so each is one dense DMA)
    xT = np.ascontiguousarray(
        x_flat.reshape(B, 2, C, 4, 128).transpose(0, 1, 4, 3, 2)).astype(bf16)
    # residual with output bias folded in
    xres = np.ascontiguousarray(x_flat + b_out[None, None, :])

    wt = w_qkv.reshape(4, 128, H, 3, D)
    wq = np.ascontiguousarray(
        wt[:, :, :, 0, :].transpose(1, 0, 2, 3).reshape(128, 4, C)).astype(bf16)
    wk = np.ascontiguousarray(
        wt[:, :, :, 1, :].transpose(1, 0, 2, 3).reshape(128, 4, C)).astype(bf16)
    wv = np.ascontiguousarray(
        wt[:, :, :, 2, :].transpose(1, 0, 2, 3).reshape(128, 4, C)).astype(bf16)
    wo = np.ascontiguousarray(
        w_out.reshape(4, 128, C).transpose(1, 0, 2)).astype(bf16)

    bt = b_qkv.reshape(H, 3, D)
    # b_col[p, mt] for head 2mt + p//64, d = p%64
    bq = np.ascontiguousarray(
        bt[:, 0, :].reshape(4, 2, D).transpose(1, 2, 0).reshape(128, 4))
    bk = np.ascontiguousarray(
        bt[:, 1, :].reshape(4, 2, D).transpose(1, 2, 0).reshape(128, 4))
    bv = np.ascontiguousarray(bt[:, 2, :].reshape(C))
    return xT, xres, wq, wk, wv, wo, bq, bk, bv


def kernel(x, w_qkv, b_qkv, w_out, b_out):
    xT, xres, wq, wk, wv, wo, bq, bk, bv = _prep_inputs(
        x, w_qkv, b_qkv, w_out, b_out)

    if "nc" not in _cache:
        _cache["nc"] = build_nc()
    nc = _cache["nc"]

    if TRACE:
        _register_ntff_hook()

    in_maps = []
    for core in range(NCORES):
        sl = slice(NB * core, NB * (core + 1))
        in_maps.append({
            "xT": xT[sl],
            "xres": xres[sl],
            "wq": wq, "wk": wk, "wv": wv, "wo": wo,
            "bq": bq, "bk": bk, "bv": bv,
        })
    res = run_bass_kernel_spmd(nc, in_maps, list(range(NCORES)), trace=TRACE)
    _cache["last_result"] = res
    y = np.concatenate([res.results[i]["y"] for i in range(NCORES)], axis=0)
    return y.reshape(B, 32, 32, C)
